# revision 24
# baseline (speedup 1.0000x reference)
"""Trainium2 Bass kernel for nn_CrossAttention (B=4, C=128, S=32, H=128, W=512).

Strategy (8 NeuronCores, SPMD single program):
  core c -> (batch b=c//2, W-half j=c%2).
  Each core: conv_block K/V over its 256-col half of y (streamed in 8
  W-tiles of 32, fused: K/V never touch DRAM), conv_block Q over x
  (duplicated per pair), attention computed per K/V column for the
  queries whose u falls in that column (host assigns queries to
  per-column slots, capacity M=16; Q gathered into slot order on device
  via one gpsimd ap_gather). Attention runs transposed ([h, slot]
  logits) in bf16 with skip-max softmax (|logit| <= ~51 empirically,
  exp stays finite in f32); normalization by the exp-sum happens on the
  host. Output: per-slot unnormalized attention vectors + sums; host
  divides and scatters to the query grid. A second tiny kernel applies
  the final 3x3 proj conv+relu (core c -> (batch, 16-row half)).

Convs run as float32r (12-bit mantissa, 1 cyc/row at free-dim>=256);
attention matmuls run bf16 (1 cyc/row at any free size).
"""
import numpy as np
import ml_dtypes

import concourse.tile as tile
from concourse import bacc, mybir
from concourse.bass_utils import run_bass_kernel_spmd

# ---- problem constants (hardcoded per spec) ----
BB, CC, S = 4, 128, 32
SS = S * S            # 1024 queries/batch
HH, WW = 128, 512
SCALE = float(32 ** -0.5)
TW = 32               # W-tile width
M = 16                # slots per column
WH = 256              # per-core W half
NSLOT = WH * M        # 4096
NTILE = WH // TW      # 8
TSLOT = TW * M        # 512 slots per tile
NCORE = 8

f32 = mybir.dt.float32
f32r = mybir.dt.float32r
bf16 = mybir.dt.bfloat16
i16 = mybir.dt.int16
AF = mybir.ActivationFunctionType
ALU = mybir.AluOpType
AX = mybir.AxisListType

USE_F32R = True

LAST_EXEC_NS = None
LAST_EXEC_NS_A = None
LAST_EXEC_NS_B = None
LAST_TRACE_A = None
LAST_TRACE_B = None

_NC_CACHE = {}

WNAMES = ["wq1", "wq2", "wk1", "wk2", "wv1", "wv2"]


def _mdt():
    return f32r if USE_F32R else f32


def _conv_groups(rows, r_grp):
    """Split `rows` into groups of r_grp (last may be smaller)."""
    out = []
    r0 = 0
    while r0 < rows:
        out.append((r0, min(r_grp, rows - r0)))
        r0 += r_grp
    return out


def _chunks(lst, n):
    for i in range(0, len(lst), n):
        yield lst[i:i + n]


def build_kernel_a(reps=1):
    mdt = _mdt()
    nc = bacc.Bacc()

    y_tiles = nc.dram_tensor("y_tiles", [NTILE, 128, 130, 36], mdt,
                             kind="ExternalInput")
    x_slab = nc.dram_tensor("x_slab", [128, 34, 34], mdt, kind="ExternalInput")
    w_dram = {n: nc.dram_tensor(n, [128, 9, 128], mdt, kind="ExternalInput")
              for n in WNAMES}
    bias6 = nc.dram_tensor("bias6", [128, 6], f32, kind="ExternalInput")
    slotq16 = nc.dram_tensor("slotq16", [128, NSLOT // 16], i16,
                             kind="ExternalInput")
    identb = nc.dram_tensor("identb", [128, 128], bf16, kind="ExternalInput")
    onesb = nc.dram_tensor("onesb", [128, 1], bf16, kind="ExternalInput")
    edge = nc.dram_tensor("edge", [128, 16], f32, kind="ExternalInput")
    a_out = nc.dram_tensor("a_out", [NTILE, 128, TSLOT], bf16,
                           kind="ExternalOutput")
    sums_out = nc.dram_tensor("sums_out", [1, NSLOT], f32,
                              kind="ExternalOutput")

    from contextlib import ExitStack
    with tile.TileContext(nc) as tc, ExitStack() as ctx:
        wpool = ctx.enter_context(tc.tile_pool(name="weights", bufs=1))
        const = ctx.enter_context(tc.tile_pool(name="const", bufs=1))
        qgp = ctx.enter_context(tc.tile_pool(name="qg", bufs=1))
        ps = ctx.enter_context(tc.tile_pool(name="ps", bufs=8, space="PSUM"))

        def psum_tile():
            return ps.tile([128, 512], f32, tag="ps", name="pst")

        # prefetch tile 0's y slab ahead of the weight DMAs
        yp = ctx.enter_context(tc.tile_pool(name="ytile", bufs=2))
        yt0 = yp.tile([128, 130, 36], mdt, tag="yt")
        nc.sync.dma_start(out=yt0[:], in_=y_tiles[0])
        xq = const.tile([128, 34, 34], mdt, tag="xq")
        nc.sync.dma_start(out=xq[:], in_=x_slab[:])
        bias_sb = const.tile([128, 6], f32, tag="bias")
        nc.sync.dma_start(out=bias_sb[:], in_=bias6[:])
        w_sb = {}
        for n in WNAMES:
            t = wpool.tile([128, 9, 128], mdt, tag=n)
            nc.sync.dma_start(out=t[:], in_=w_dram[n][:])
            w_sb[n] = t
        sq_sb = const.tile([128, NSLOT // 16], i16, tag="sq")
        nc.sync.dma_start(out=sq_sb[:], in_=slotq16[:])
        ident_sb = const.tile([128, 128], bf16, tag="identb")
        nc.sync.dma_start(out=ident_sb[:], in_=identb[:])
        ones_sb = const.tile([128, 1], bf16, tag="onesb")
        nc.sync.dma_start(out=ones_sb[:], in_=onesb[:])
        edge_sb = const.tile([128, 16], f32, tag="edge")
        nc.sync.dma_start(out=edge_sb[:], in_=edge[:])
        zeros34 = const.tile([128, 34], f32, tag="zeros34")
        nc.vector.memset(zeros34[:], 0.0)
        sums_sb = const.tile([1, NSLOT], f32, tag="sums")

        def zfill(ap):
            # zero-fill an f32r view via DVE copy (memset can't emit f32r)
            nc.vector.tensor_copy(out=ap, in_=zeros34[:, :ap.free_size()])

        # Qg (f32, bitcast to f32r at use) and q2 live in the persistent
        # pool: the gpsimd gather runs long (~100us) concurrent with the
        # conv loop, so nothing in the reused-scope may alias them.
        Qg = qgp.tile([128, NSLOT], f32, tag="Qg")
        q2 = qgp.tile([128, 1024], f32, tag="q2")

        # ---------------- Q path + slot gather ----------------
        with tc.tile_pool(name="qtmp", bufs=1) as qtmp:
            q1 = qtmp.tile([128, 34, 34], mdt, tag="q1")
            q1f = q1[:].rearrange("p a b -> p (a b)")
            zfill(q1f[:, 0:34])                              # row 0
            zfill(q1f[:, 33 * 34:34 * 34])                   # row 33
            zfill(q1[:, :, 0:1].rearrange("p a b -> p (a b)"))    # col 0
            zfill(q1[:, :, 33:34].rearrange("p a b -> p (a b)"))  # col 33
            # conv1-Q: valid 32x32 -> q1[1:33, 1:33]
            for r0 in (0, 16):
                pt = psum_tile()
                for tap in range(9):
                    dy, dx = divmod(tap, 3)
                    nc.tensor.matmul(pt[:, :512], w_sb["wq1"][:, tap, :],
                                     xq[:, r0 + dy:r0 + dy + 16, dx:dx + 32],
                                     start=(tap == 0), stop=(tap == 8))
                nc.scalar.activation(out=q1[:, 1 + r0:1 + r0 + 16, 1:33],
                                     in_=pt[:, :512].rearrange(
                                         "p (a b) -> p a b", a=16),
                                     func=AF.Relu, bias=bias_sb[:, 0:1],
                                     scale=1.0)
            # conv2-Q -> Q (128, 1024) f32, scaled by SCALE
            for r0 in (0, 16):
                pt = psum_tile()
                for tap in range(9):
                    dy, dx = divmod(tap, 3)
                    nc.tensor.matmul(pt[:, :512], w_sb["wq2"][:, tap, :],
                                     q1[:, r0 + dy:r0 + dy + 16, dx:dx + 32],
                                     start=(tap == 0), stop=(tap == 8))
                nc.scalar.activation(out=q2[:, r0 * 32:(r0 + 16) * 32],
                                     in_=pt[:, :512], func=AF.Identity,
                                     bias=bias_sb[:, 1:2], scale=SCALE)
        # gather Q columns into slot order (gpsimd, overlaps conv loop)
        nc.gpsimd.ap_gather(out_ap=Qg[:], in_ap=q2[:], idxs_ap=sq_sb[:],
                            channels=128, num_elems=SS, d=1,
                            num_idxs=NSLOT)

        # ---------------- main loop over W-tiles ----------------
        c1p = ctx.enter_context(tc.tile_pool(name="c1", bufs=1))
        k2p = ctx.enter_context(tc.tile_pool(name="k2", bufs=2))
        v2p = ctx.enter_context(tc.tile_pool(name="v2", bufs=2))
        v2tp = ctx.enter_context(tc.tile_pool(name="v2t", bufs=2))
        ptp = ctx.enter_context(tc.tile_pool(name="pt", bufs=2))
        abp = ctx.enter_context(tc.tile_pool(name="ab", bufs=2))

        g1 = _conv_groups(128, 15)   # conv1 valid rows (h 0..127)
        g2 = _conv_groups(128, 16)   # conv2 rows

        def attn_head(t, k2, v2):
            # logits^T: [h, slot] per column; one psum bank holds the
            # whole tile's 512 slots. f32 logits (LD-bound, 16 rows).
            ps_s = psum_tile()
            for wl in range(TW):
                s0 = wl * M
                qslice = Qg[:, t * TSLOT + s0:t * TSLOT + s0 + M]
                nc.tensor.matmul(ps_s[:, s0:s0 + M], k2[:, :, wl], qslice,
                                 start=True, stop=True)
            # skip-max softmax numerator: p~ = exp(logit), bf16.
            # Runs on Scalar during the next tile's convs.
            pT = ptp.tile([128, TSLOT], bf16, tag="pT")
            nc.scalar.activation(out=pT[:], in_=ps_s[:, :TSLOT], func=AF.Exp,
                                 bias=0.0, scale=1.0)
            # V^T per column: [h, e] via PE transpose (bf16); fills the
            # tensor queue while exp(t) runs on Scalar.
            v2t = v2tp.tile([128, 32, 128], bf16, tag="v2t")
            for wl in range(TW):
                ptr = psum_tile()
                ptrv = ptr[:, 0:64].bitcast(bf16)
                nc.tensor.transpose(ptrv[:, :128], v2[:, :, wl], ident_sb[:])
                nc.vector.tensor_copy(out=v2t[:, wl, :], in_=ptrv[:, :128])
            return t, pT, v2t

        def attn_tail(t, pT, v2t):
            # exp-sums per slot: ones^T @ p~
            ps_m = psum_tile()
            nc.tensor.matmul(ps_m[0:1, :TSLOT], ones_sb[:], pT[:],
                             start=True, stop=True)
            nc.vector.tensor_copy(out=sums_sb[:, t * TSLOT:(t + 1) * TSLOT],
                                  in_=ps_m[0:1, :TSLOT])
            # a^T = V^T @ p~ : [e, slot], unnormalized
            ps_a = psum_tile()
            for wl in range(TW):
                s0 = wl * M
                nc.tensor.matmul(ps_a[:, s0:s0 + M], v2t[:, wl, :],
                                 pT[:, s0:s0 + M], start=True, stop=True)
            a_sb = abp.tile([128, TSLOT], bf16, tag="a")
            nc.vector.tensor_copy(out=a_sb[:], in_=ps_a[:, :TSLOT])
            nc.sync.dma_start(out=a_out[t], in_=a_sb[:])

        prev = None
        first = True
        for t in list(range(NTILE)) * reps:
            if first:
                yt = yt0
                first = False
            else:
                yt = yp.tile([128, 130, 36], mdt, tag="yt")
                nc.sync.dma_start(out=yt[:], in_=y_tiles[t])
            k2 = v2 = None
            for (w1n, w2n, b1i, b2i, kind) in [
                    ("wk1", "wk2", 2, 3, "K"), ("wv1", "wv2", 4, 5, "V")]:
                c1 = c1p.tile([128, 130, 34], mdt, tag="c1")
                c1f = c1[:].rearrange("p a b -> p (a b)")
                zfill(c1f[:, 0:34])                  # h = -1 (conv2 zero pad)
                zfill(c1f[:, 129 * 34:130 * 34])     # h = 128
                for sg in _chunks(g1, 4):
                    pts = [psum_tile() for _ in sg]
                    for tap in range(9):
                        dy, dx = divmod(tap, 3)
                        for (r0, R), pt in zip(sg, pts):
                            nc.tensor.matmul(
                                pt[:, :R * 34], w_sb[w1n][:, tap, :],
                                yt[:, r0 + dy:r0 + dy + R, dx:dx + 34],
                                start=(tap == 0), stop=(tap == 8))
                    for (r0, R), pt in zip(sg, pts):
                        nc.scalar.activation(
                            out=c1f[:, (1 + r0) * 34:(1 + r0 + R) * 34],
                            in_=pt[:, :R * 34], func=AF.Relu,
                            bias=bias_sb[:, b1i:b1i + 1], scale=1.0)
                # zero conv1 halo cols outside the global image (data mask)
                nc.vector.tensor_scalar_mul(c1[:, :, 0:1], c1[:, :, 0:1],
                                            edge_sb[:, 2 * t:2 * t + 1])
                nc.vector.tensor_scalar_mul(c1[:, :, 33:34], c1[:, :, 33:34],
                                            edge_sb[:, 2 * t + 1:2 * t + 2])
                cdt = f32 if kind == "K" else bf16
                pool2 = k2p if kind == "K" else v2p
                cv2 = pool2.tile([128, 128, 32], cdt, tag="cv2" + kind)
                cv2f = cv2[:].rearrange("p a b -> p (a b)")
                for sg in _chunks(g2, 4):
                    pts = [psum_tile() for _ in sg]
                    for tap in range(9):
                        dy, dx = divmod(tap, 3)
                        for (r0, R), pt in zip(sg, pts):
                            nc.tensor.matmul(
                                pt[:, :R * 32], w_sb[w2n][:, tap, :],
                                c1[:, r0 + dy:r0 + dy + R, dx:dx + 32],
                                start=(tap == 0), stop=(tap == 8))
                    for (r0, R), pt in zip(sg, pts):
                        nc.scalar.activation(
                            out=cv2f[:, r0 * 32:(r0 + R) * 32],
                            in_=pt[:, :R * 32], func=AF.Identity,
                            bias=bias_sb[:, b2i:b2i + 1], scale=1.0)
                if kind == "K":
                    k2 = cv2
                else:
                    v2 = cv2
            if prev is not None:
                attn_tail(*prev)
            prev = attn_head(t, k2, v2)
        attn_tail(*prev)

        nc.sync.dma_start(out=sums_out[:], in_=sums_sb[:])
    nc.compile()
    return nc


def build_kernel_b():
    mdt = _mdt()
    nc = bacc.Bacc()
    a_slab = nc.dram_tensor("a_slab", [128, 18, 34], mdt, kind="ExternalInput")
    wp = nc.dram_tensor("wp", [128, 9, 128], mdt, kind="ExternalInput")
    bp = nc.dram_tensor("bp", [128, 1], f32, kind="ExternalInput")
    z_out = nc.dram_tensor("z_out", [128, 512], f32, kind="ExternalOutput")

    with tile.TileContext(nc) as tc:
        with tc.tile_pool(name="sb", bufs=1) as sb, \
             tc.tile_pool(name="ps", bufs=2, space="PSUM") as ps:
            a_sb = sb.tile([128, 18, 34], mdt)
            nc.sync.dma_start(out=a_sb[:], in_=a_slab[:])
            wp_sb = sb.tile([128, 9, 128], mdt)
            nc.sync.dma_start(out=wp_sb[:], in_=wp[:])
            bp_sb = sb.tile([128, 1], f32)
            nc.sync.dma_start(out=bp_sb[:], in_=bp[:])
            pt = ps.tile([128, 512], f32)
            for tap in range(9):
                dy, dx = divmod(tap, 3)
                nc.tensor.matmul(pt[:], wp_sb[:, tap, :],
                                 a_sb[:, dy:dy + 16, dx:dx + 32],
                                 start=(tap == 0), stop=(tap == 8))
            z_sb = sb.tile([128, 512], f32)
            nc.scalar.activation(out=z_sb[:], in_=pt[:], func=AF.Relu,
                                 bias=bp_sb[:, 0:1], scale=1.0)
            nc.sync.dma_start(out=z_out[:], in_=z_sb[:])
    nc.compile()
    return nc


def _round12(a):
    if not USE_F32R:
        return np.ascontiguousarray(a, np.float32)
    b = np.ascontiguousarray(a, np.float32).view(np.uint32)
    b = (b + np.uint32(0x400)) & np.uint32(0xFFFFF800)
    return b.view(np.float32)


def _get_nc(which):
    key = (which, USE_F32R, M)
    if key not in _NC_CACHE:
        _NC_CACHE[key] = (build_kernel_a() if which == "a"
                          else build_kernel_b())
    return _NC_CACHE[key]


def _prep_core_a(xr, yr, uc, wt, bias6, b, j):
    """Per-core host prep. xr/yr pre-rounded full arrays."""
    y = yr[b]                      # (128, 128, 512)
    x = xr[b]                      # (128, 32, 32)
    u = uc[b].reshape(SS)          # int64 in [0, 512)

    x_slab = np.zeros((128, 34, 34), np.float32)
    x_slab[:, 1:33, 1:33] = x

    y_slab = np.zeros((128, 130, 260), np.float32)
    lo, hi = WH * j - 2, WH * j + WH + 2
    glo, ghi = max(lo, 0), min(hi, WW)
    y_slab[:, 1:129, (glo - lo):(ghi - lo)] = y[:, :, glo:ghi]
    y_tiles = np.stack([y_slab[:, :, TW * t:TW * t + 36]
                        for t in range(NTILE)])

    local = u - WH * j
    mask = (local >= 0) & (local < WH)
    slotq = np.full((NSLOT,), -1, np.int64)   # query index per slot
    counts = np.zeros(WH, np.int64)
    for q in range(SS):
        if mask[q]:
            w = int(local[q])
            r = counts[w]
            assert r < M, f"column {w} overflows {M} slots"
            slotq[w * M + r] = q
            counts[w] += 1

    # gpsimd ap_gather wrapped index layout: idx for slot s lives at
    # [s % 16, s // 16], replicated across the 8 gpsimd cores.
    idx16 = np.where(slotq >= 0, slotq, 0).astype(np.int16)
    wrapped = idx16.reshape(NSLOT // 16, 16).T          # (16, NSLOT//16)
    slotq16 = np.tile(wrapped, (8, 1))                  # (128, NSLOT//16)

    edge = np.ones((128, 16), np.float32)
    if j == 0:
        edge[:, 0] = 0.0        # tile 0, col0 -> global col -1
    else:
        edge[:, 2 * (NTILE - 1) + 1] = 0.0   # last tile col33 -> global 512

    in_map = {
        "y_tiles": y_tiles,
        "x_slab": x_slab,
        "bias6": bias6,
        "slotq16": slotq16,
        "identb": np.eye(128, dtype=np.float32).astype(ml_dtypes.bfloat16),
        "onesb": np.ones((128, 1), ml_dtypes.bfloat16),
        "edge": edge,
    }
    in_map.update(wt)
    return in_map, slotq


def kernel(x, y, u, q_w1, q_b1, q_w2, q_b2, k_w1, k_b1, k_w2, k_b2,
           v_w1, v_b1, v_w2, v_b2, proj_w, proj_b):
    x = np.asarray(x, np.float32)
    y = np.asarray(y, np.float32)
    u_in = np.asarray(u)
    uc = np.clip(u_in, 0, WW - 1).astype(np.int64)

    xr, yr = _round12(x), _round12(y)
    wsrc = {"wq1": q_w1, "wq2": q_w2, "wk1": k_w1, "wk2": k_w2,
            "wv1": v_w1, "wv2": v_w2}
    wt = {n: _round12(np.asarray(w, np.float32)
                      .transpose(1, 2, 3, 0).reshape(128, 9, 128))
          for n, w in wsrc.items()}
    bias6 = np.stack([
        np.asarray(q_b1, np.float32),
        np.asarray(q_b2, np.float32) * np.float32(SCALE),
        np.asarray(k_b1, np.float32), np.asarray(k_b2, np.float32),
        np.asarray(v_b1, np.float32), np.asarray(v_b2, np.float32),
    ], axis=1)                     # (128, 6)

    in_maps, slot_maps = [], []
    for c in range(NCORE):
        im, sq = _prep_core_a(xr, yr, uc, wt, bias6, c // 2, c % 2)
        in_maps.append(im)
        slot_maps.append(sq)

    nc_a = _get_nc("a")
    res_a = run_bass_kernel_spmd(nc_a, in_maps, list(range(NCORE)))
    global LAST_EXEC_NS_A, LAST_TRACE_A
    LAST_EXEC_NS_A = res_a.exec_time_ns
    LAST_TRACE_A = res_a.instructions_and_trace

    a_full = np.zeros((BB, SS, 128), np.float32)
    for c in range(NCORE):
        # a_out: [NTILE, e, TSLOT] -> [slot, e]; divide by exp-sums
        aT = np.asarray(res_a.results[c]["a_out"], np.float32)
        flat = aT.transpose(0, 2, 1).reshape(NSLOT, 128)
        sums = np.asarray(res_a.results[c]["sums_out"], np.float32)
        sums = sums.reshape(NSLOT)
        sq = slot_maps[c]
        valid = sq >= 0
        a_full[c // 2][sq[valid]] = flat[valid] / sums[valid, None]
    a_img = a_full.transpose(0, 2, 1).reshape(BB, 128, S, S)

    wpr = _round12(np.asarray(proj_w, np.float32)
                   .transpose(1, 2, 3, 0).reshape(128, 9, 128))
    bpr = np.asarray(proj_b, np.float32).reshape(128, 1)
    in_maps_b = []
    for c in range(NCORE):
        b, rh = c // 2, c % 2
        a_slab = np.zeros((128, 18, 34), np.float32)
        r0 = 16 * rh
        rlo, rhi = max(r0 - 1, 0), min(r0 + 17, S)
        a_slab[:, (rlo - (r0 - 1)):(rhi - (r0 - 1)), 1:33] = \
            _round12(a_img[b, :, rlo:rhi, :])
        in_maps_b.append({"a_slab": a_slab, "wp": wpr, "bp": bpr})

    nc_b = _get_nc("b")
    res_b = run_bass_kernel_spmd(nc_b, in_maps_b, list(range(NCORE)))
    global LAST_EXEC_NS_B, LAST_TRACE_B, LAST_EXEC_NS
    LAST_EXEC_NS_B = res_b.exec_time_ns
    LAST_TRACE_B = res_b.instructions_and_trace
    if LAST_EXEC_NS_A is not None and LAST_EXEC_NS_B is not None:
        LAST_EXEC_NS = LAST_EXEC_NS_A + LAST_EXEC_NS_B
    return z_from_b(res_b)


def z_from_b(res_b):
    z = np.zeros((BB, 128, S, S), np.float32)
    for c in range(NCORE):
        b, rh = c // 2, c % 2
        z[b, :, 16 * rh:16 * rh + 16, :] = \
            res_b.results[c]["z_out"].reshape(128, 16, 32)
    return z


# revision 26
# speedup vs baseline: 1.2395x; 1.2395x over previous
"""Trainium2 Bass kernel for nn_CrossAttention (B=4, C=128, S=32, H=128, W=512).

Strategy (8 NeuronCores, SPMD single program):
  core c -> (batch b=c//2, W-half j=c%2).
  Each core: conv_block K/V over its 256-col half of y (streamed in 8
  W-tiles of 32, fused: K/V never touch DRAM), conv_block Q over x
  (duplicated per pair), attention computed per K/V column for the
  queries whose u falls in that column (host assigns queries to
  per-column slots, capacity M=16; Q gathered into slot order on device
  via one gpsimd ap_gather). Attention runs transposed ([h, slot]
  logits) in bf16 with skip-max softmax (|logit| <= ~51 empirically,
  exp stays finite in f32); normalization by the exp-sum happens on the
  host. Output: per-slot unnormalized attention vectors + sums; host
  divides and scatters to the query grid. A second tiny kernel applies
  the final 3x3 proj conv+relu (core c -> (batch, 16-row half)).

Convs run as float32r (12-bit mantissa, 1 cyc/row at free-dim>=256);
attention matmuls run bf16 (1 cyc/row at any free size).
"""
import numpy as np
import ml_dtypes

import concourse.tile as tile
from concourse import bacc, mybir
from concourse.bass_utils import run_bass_kernel_spmd

# ---- problem constants (hardcoded per spec) ----
BB, CC, S = 4, 128, 32
SS = S * S            # 1024 queries/batch
HH, WW = 128, 512
SCALE = float(32 ** -0.5)
TW = 32               # W-tile width
M = 16                # slots per column
WH = 256              # per-core W half
NSLOT = WH * M        # 4096
NTILE = WH // TW      # 8
TSLOT = TW * M        # 512 slots per tile
NCORE = 8

f32 = mybir.dt.float32
f32r = mybir.dt.float32r
bf16 = mybir.dt.bfloat16
i16 = mybir.dt.int16
AF = mybir.ActivationFunctionType
ALU = mybir.AluOpType
AX = mybir.AxisListType

USE_F32R = True

LAST_EXEC_NS = None
LAST_EXEC_NS_A = None
LAST_EXEC_NS_B = None
LAST_TRACE_A = None
LAST_TRACE_B = None

_NC_CACHE = {}

WNAMES = ["wq1", "wq2", "wk1", "wk2", "wv1", "wv2"]


def _mdt():
    return f32r if USE_F32R else f32


def _conv_groups(rows, r_grp):
    """Split `rows` into groups of r_grp (last may be smaller)."""
    out = []
    r0 = 0
    while r0 < rows:
        out.append((r0, min(r_grp, rows - r0)))
        r0 += r_grp
    return out


def _chunks(lst, n):
    for i in range(0, len(lst), n):
        yield lst[i:i + n]


def build_kernel_a(reps=1):
    mdt = _mdt()
    nc = bacc.Bacc()

    y_tiles = nc.dram_tensor("y_tiles", [NTILE, 128, 130, 36], mdt,
                             kind="ExternalInput")
    x_slab = nc.dram_tensor("x_slab", [128, 34, 34], mdt, kind="ExternalInput")
    w_dram = {n: nc.dram_tensor(n, [128, 9, 128], mdt, kind="ExternalInput")
              for n in WNAMES}
    bias6 = nc.dram_tensor("bias6", [128, 6], f32, kind="ExternalInput")
    slotq16 = nc.dram_tensor("slotq16", [128, NSLOT // 16], i16,
                             kind="ExternalInput")
    identb = nc.dram_tensor("identb", [128, 128], bf16, kind="ExternalInput")
    onesb = nc.dram_tensor("onesb", [128, 1], bf16, kind="ExternalInput")
    edge = nc.dram_tensor("edge", [128, 16], f32, kind="ExternalInput")
    a_out = nc.dram_tensor("a_out", [NTILE, 128, TSLOT], bf16,
                           kind="ExternalOutput")
    sums_out = nc.dram_tensor("sums_out", [1, NSLOT], f32,
                              kind="ExternalOutput")

    from contextlib import ExitStack
    with tile.TileContext(nc) as tc, ExitStack() as ctx:
        wpool = ctx.enter_context(tc.tile_pool(name="weights", bufs=1))
        const = ctx.enter_context(tc.tile_pool(name="const", bufs=1))
        qgp = ctx.enter_context(tc.tile_pool(name="qg", bufs=1))
        ps = ctx.enter_context(tc.tile_pool(name="ps", bufs=8, space="PSUM"))

        def psum_tile():
            return ps.tile([128, 512], f32, tag="ps", name="pst")

        # prefetch tile 0's y slab ahead of the weight DMAs
        yp = ctx.enter_context(tc.tile_pool(name="ytile", bufs=2))
        yt0 = yp.tile([128, 130, 36], mdt, tag="yt")
        nc.sync.dma_start(out=yt0[:], in_=y_tiles[0])
        xq = const.tile([128, 34, 34], mdt, tag="xq")
        nc.sync.dma_start(out=xq[:], in_=x_slab[:])
        bias_sb = const.tile([128, 6], f32, tag="bias")
        nc.sync.dma_start(out=bias_sb[:], in_=bias6[:])
        w_sb = {}
        for n in WNAMES:
            t = wpool.tile([128, 9, 128], mdt, tag=n)
            nc.sync.dma_start(out=t[:], in_=w_dram[n][:])
            w_sb[n] = t
        sq_sb = const.tile([128, NSLOT // 16], i16, tag="sq")
        nc.sync.dma_start(out=sq_sb[:], in_=slotq16[:])
        ident_sb = const.tile([128, 128], bf16, tag="identb")
        nc.sync.dma_start(out=ident_sb[:], in_=identb[:])
        ones_sb = const.tile([128, 1], bf16, tag="onesb")
        nc.sync.dma_start(out=ones_sb[:], in_=onesb[:])
        edge_sb = const.tile([128, 16], f32, tag="edge")
        nc.sync.dma_start(out=edge_sb[:], in_=edge[:])
        zeros34 = const.tile([128, 34], f32, tag="zeros34")
        nc.vector.memset(zeros34[:], 0.0)
        sums_sb = const.tile([1, NSLOT], f32, tag="sums")

        def zfill(ap):
            # zero-fill an f32r view via DVE copy (memset can't emit f32r)
            nc.vector.tensor_copy(out=ap, in_=zeros34[:, :ap.free_size()])

        # Qg (f32, bitcast to f32r at use) and q2 live in the persistent
        # pool: the gpsimd gather runs long (~100us) concurrent with the
        # conv loop, so nothing in the reused-scope may alias them.
        Qg = qgp.tile([128, NSLOT], f32, tag="Qg")
        q2 = qgp.tile([128, 1024], f32, tag="q2")

        # ---------------- Q path + slot gather ----------------
        with tc.tile_pool(name="qtmp", bufs=1) as qtmp:
            q1 = qtmp.tile([128, 34, 34], mdt, tag="q1")
            q1f = q1[:].rearrange("p a b -> p (a b)")
            zfill(q1f[:, 0:34])                              # row 0
            zfill(q1f[:, 33 * 34:34 * 34])                   # row 33
            zfill(q1[:, :, 0:1].rearrange("p a b -> p (a b)"))    # col 0
            zfill(q1[:, :, 33:34].rearrange("p a b -> p (a b)"))  # col 33
            # conv1-Q: valid 32x32 -> q1[1:33, 1:33]
            for r0 in (0, 16):
                pt = psum_tile()
                for tap in range(9):
                    dy, dx = divmod(tap, 3)
                    nc.tensor.matmul(pt[:, :512], w_sb["wq1"][:, tap, :],
                                     xq[:, r0 + dy:r0 + dy + 16, dx:dx + 32],
                                     start=(tap == 0), stop=(tap == 8))
                nc.scalar.activation(out=q1[:, 1 + r0:1 + r0 + 16, 1:33],
                                     in_=pt[:, :512].rearrange(
                                         "p (a b) -> p a b", a=16),
                                     func=AF.Relu, bias=bias_sb[:, 0:1],
                                     scale=1.0)
            # conv2-Q -> Q (128, 1024) f32, scaled by SCALE
            for r0 in (0, 16):
                pt = psum_tile()
                for tap in range(9):
                    dy, dx = divmod(tap, 3)
                    nc.tensor.matmul(pt[:, :512], w_sb["wq2"][:, tap, :],
                                     q1[:, r0 + dy:r0 + dy + 16, dx:dx + 32],
                                     start=(tap == 0), stop=(tap == 8))
                nc.scalar.activation(out=q2[:, r0 * 32:(r0 + 16) * 32],
                                     in_=pt[:, :512], func=AF.Identity,
                                     bias=bias_sb[:, 1:2], scale=SCALE)
        # gather Q columns into slot order (gpsimd, overlaps conv loop)
        nc.gpsimd.ap_gather(out_ap=Qg[:], in_ap=q2[:], idxs_ap=sq_sb[:],
                            channels=128, num_elems=SS, d=1,
                            num_idxs=NSLOT)

        # ---------------- main loop over W-tiles ----------------
        c1p = ctx.enter_context(tc.tile_pool(name="c1", bufs=1))
        k2p = ctx.enter_context(tc.tile_pool(name="k2", bufs=2))
        v2p = ctx.enter_context(tc.tile_pool(name="v2", bufs=2))
        v2tp = ctx.enter_context(tc.tile_pool(name="v2t", bufs=2))
        ptp = ctx.enter_context(tc.tile_pool(name="pt", bufs=2))
        abp = ctx.enter_context(tc.tile_pool(name="ab", bufs=2))

        g1 = _conv_groups(128, 15)   # conv1 valid rows (h 0..127)
        g2 = _conv_groups(128, 16)   # conv2 rows

        def attn_head(t, k2, v2):
            # logits^T: [h, slot] per column; one psum bank holds the
            # whole tile's 512 slots. f32 logits (LD-bound, 16 rows).
            ps_s = psum_tile()
            for wl in range(TW):
                s0 = wl * M
                qslice = Qg[:, t * TSLOT + s0:t * TSLOT + s0 + M]
                nc.tensor.matmul(ps_s[:, s0:s0 + M], k2[:, :, wl], qslice,
                                 start=True, stop=True)
            # skip-max softmax numerator: p~ = exp(logit), bf16.
            # Runs on Scalar during the next tile's convs.
            pT = ptp.tile([128, TSLOT], bf16, tag="pT")
            nc.scalar.activation(out=pT[:], in_=ps_s[:, :TSLOT], func=AF.Exp,
                                 bias=0.0, scale=1.0)
            # V^T per column: [h, e] via PE transpose (bf16); fills the
            # tensor queue while exp(t) runs on Scalar.
            v2t = v2tp.tile([128, 32, 128], bf16, tag="v2t")
            for wl in range(TW):
                ptr = psum_tile()
                ptrv = ptr[:, 0:64].bitcast(bf16)
                nc.tensor.transpose(ptrv[:, :128], v2[:, :, wl], ident_sb[:])
                nc.vector.tensor_copy(out=v2t[:, wl, :], in_=ptrv[:, :128])
            return t, pT, v2t

        def attn_tail(t, pT, v2t):
            # exp-sums per slot: ones^T @ p~ (exp covered by the V2T block)
            ps_m = psum_tile()
            nc.tensor.matmul(ps_m[0:1, :TSLOT], ones_sb[:], pT[:],
                             start=True, stop=True)
            nc.vector.tensor_copy(out=sums_sb[:, t * TSLOT:(t + 1) * TSLOT],
                                  in_=ps_m[0:1, :TSLOT])
            # a^T = V^T @ p~ : [e, slot], unnormalized
            ps_a = psum_tile()
            for wl in range(TW):
                s0 = wl * M
                nc.tensor.matmul(ps_a[:, s0:s0 + M], v2t[:, wl, :],
                                 pT[:, s0:s0 + M], start=True, stop=True)
            a_sb = abp.tile([128, TSLOT], bf16, tag="a")
            nc.vector.tensor_copy(out=a_sb[:], in_=ps_a[:, :TSLOT])
            nc.sync.dma_start(out=a_out[t], in_=a_sb[:])

        prev = None
        first = True
        for t in list(range(NTILE)) * reps:
            if first:
                yt = yt0
                first = False
            else:
                yt = yp.tile([128, 130, 36], mdt, tag="yt")
                nc.sync.dma_start(out=yt[:], in_=y_tiles[t])
            k2 = v2 = None
            for (w1n, w2n, b1i, b2i, kind) in [
                    ("wk1", "wk2", 2, 3, "K"), ("wv1", "wv2", 4, 5, "V")]:
                c1 = c1p.tile([128, 130, 34], mdt, tag="c1")
                c1f = c1[:].rearrange("p a b -> p (a b)")
                zfill(c1f[:, 0:34])                  # h = -1 (conv2 zero pad)
                zfill(c1f[:, 129 * 34:130 * 34])     # h = 128
                for sg in _chunks(g1, 4):
                    pts = [psum_tile() for _ in sg]
                    for tap in range(9):
                        dy, dx = divmod(tap, 3)
                        for (r0, R), pt in zip(sg, pts):
                            nc.tensor.matmul(
                                pt[:, :R * 34], w_sb[w1n][:, tap, :],
                                yt[:, r0 + dy:r0 + dy + R, dx:dx + 34],
                                start=(tap == 0), stop=(tap == 8))
                    for (r0, R), pt in zip(sg, pts):
                        nc.scalar.activation(
                            out=c1f[:, (1 + r0) * 34:(1 + r0 + R) * 34],
                            in_=pt[:, :R * 34], func=AF.Relu,
                            bias=bias_sb[:, b1i:b1i + 1], scale=1.0)
                # zero conv1 halo cols outside the global image (data mask)
                nc.vector.tensor_scalar_mul(c1[:, :, 0:1], c1[:, :, 0:1],
                                            edge_sb[:, 2 * t:2 * t + 1])
                nc.vector.tensor_scalar_mul(c1[:, :, 33:34], c1[:, :, 33:34],
                                            edge_sb[:, 2 * t + 1:2 * t + 2])
                cdt = f32 if kind == "K" else bf16
                pool2 = k2p if kind == "K" else v2p
                cv2 = pool2.tile([128, 128, 32], cdt, tag="cv2" + kind)
                cv2f = cv2[:].rearrange("p a b -> p (a b)")
                for sg in _chunks(g2, 4):
                    pts = [psum_tile() for _ in sg]
                    for tap in range(9):
                        dy, dx = divmod(tap, 3)
                        for (r0, R), pt in zip(sg, pts):
                            nc.tensor.matmul(
                                pt[:, :R * 32], w_sb[w2n][:, tap, :],
                                c1[:, r0 + dy:r0 + dy + R, dx:dx + 32],
                                start=(tap == 0), stop=(tap == 8))
                    for (r0, R), pt in zip(sg, pts):
                        nc.scalar.activation(
                            out=cv2f[:, r0 * 32:(r0 + R) * 32],
                            in_=pt[:, :R * 32], func=AF.Identity,
                            bias=bias_sb[:, b2i:b2i + 1], scale=1.0)
                if kind == "K":
                    k2 = cv2
                else:
                    v2 = cv2
            if prev is not None:
                # attention for the PREVIOUS tile: its QK sits behind this
                # tile's convs, covering the slow Qg gather at startup and
                # giving exp a full V2T block of Scalar time.
                attn_tail(*attn_head(*prev))
            prev = (t, k2, v2)
        attn_tail(*attn_head(*prev))

        nc.sync.dma_start(out=sums_out[:], in_=sums_sb[:])
    nc.compile()
    return nc


def build_kernel_b():
    mdt = _mdt()
    nc = bacc.Bacc()
    a_slab = nc.dram_tensor("a_slab", [128, 18, 34], mdt, kind="ExternalInput")
    wp = nc.dram_tensor("wp", [128, 9, 128], mdt, kind="ExternalInput")
    bp = nc.dram_tensor("bp", [128, 1], f32, kind="ExternalInput")
    z_out = nc.dram_tensor("z_out", [128, 512], f32, kind="ExternalOutput")

    with tile.TileContext(nc) as tc:
        with tc.tile_pool(name="sb", bufs=1) as sb, \
             tc.tile_pool(name="ps", bufs=2, space="PSUM") as ps:
            a_sb = sb.tile([128, 18, 34], mdt)
            nc.sync.dma_start(out=a_sb[:], in_=a_slab[:])
            wp_sb = sb.tile([128, 9, 128], mdt)
            nc.sync.dma_start(out=wp_sb[:], in_=wp[:])
            bp_sb = sb.tile([128, 1], f32)
            nc.sync.dma_start(out=bp_sb[:], in_=bp[:])
            pt = ps.tile([128, 512], f32)
            for tap in range(9):
                dy, dx = divmod(tap, 3)
                nc.tensor.matmul(pt[:], wp_sb[:, tap, :],
                                 a_sb[:, dy:dy + 16, dx:dx + 32],
                                 start=(tap == 0), stop=(tap == 8))
            z_sb = sb.tile([128, 512], f32)
            nc.scalar.activation(out=z_sb[:], in_=pt[:], func=AF.Relu,
                                 bias=bp_sb[:, 0:1], scale=1.0)
            nc.sync.dma_start(out=z_out[:], in_=z_sb[:])
    nc.compile()
    return nc


def _round12(a):
    if not USE_F32R:
        return np.ascontiguousarray(a, np.float32)
    b = np.ascontiguousarray(a, np.float32).view(np.uint32)
    b = (b + np.uint32(0x400)) & np.uint32(0xFFFFF800)
    return b.view(np.float32)


def _get_nc(which):
    key = (which, USE_F32R, M)
    if key not in _NC_CACHE:
        _NC_CACHE[key] = (build_kernel_a() if which == "a"
                          else build_kernel_b())
    return _NC_CACHE[key]


def _prep_core_a(xr, yr, uc, wt, bias6, b, j):
    """Per-core host prep. xr/yr pre-rounded full arrays."""
    y = yr[b]                      # (128, 128, 512)
    x = xr[b]                      # (128, 32, 32)
    u = uc[b].reshape(SS)          # int64 in [0, 512)

    x_slab = np.zeros((128, 34, 34), np.float32)
    x_slab[:, 1:33, 1:33] = x

    y_slab = np.zeros((128, 130, 260), np.float32)
    lo, hi = WH * j - 2, WH * j + WH + 2
    glo, ghi = max(lo, 0), min(hi, WW)
    y_slab[:, 1:129, (glo - lo):(ghi - lo)] = y[:, :, glo:ghi]
    y_tiles = np.stack([y_slab[:, :, TW * t:TW * t + 36]
                        for t in range(NTILE)])

    local = u - WH * j
    mask = (local >= 0) & (local < WH)
    slotq = np.full((NSLOT,), -1, np.int64)   # query index per slot
    counts = np.zeros(WH, np.int64)
    for q in range(SS):
        if mask[q]:
            w = int(local[q])
            r = counts[w]
            assert r < M, f"column {w} overflows {M} slots"
            slotq[w * M + r] = q
            counts[w] += 1

    # gpsimd ap_gather wrapped index layout: idx for slot s lives at
    # [s % 16, s // 16], replicated across the 8 gpsimd cores.
    idx16 = np.where(slotq >= 0, slotq, 0).astype(np.int16)
    wrapped = idx16.reshape(NSLOT // 16, 16).T          # (16, NSLOT//16)
    slotq16 = np.tile(wrapped, (8, 1))                  # (128, NSLOT//16)

    edge = np.ones((128, 16), np.float32)
    if j == 0:
        edge[:, 0] = 0.0        # tile 0, col0 -> global col -1
    else:
        edge[:, 2 * (NTILE - 1) + 1] = 0.0   # last tile col33 -> global 512

    in_map = {
        "y_tiles": y_tiles,
        "x_slab": x_slab,
        "bias6": bias6,
        "slotq16": slotq16,
        "identb": np.eye(128, dtype=np.float32).astype(ml_dtypes.bfloat16),
        "onesb": np.ones((128, 1), ml_dtypes.bfloat16),
        "edge": edge,
    }
    in_map.update(wt)
    return in_map, slotq


def kernel(x, y, u, q_w1, q_b1, q_w2, q_b2, k_w1, k_b1, k_w2, k_b2,
           v_w1, v_b1, v_w2, v_b2, proj_w, proj_b):
    x = np.asarray(x, np.float32)
    y = np.asarray(y, np.float32)
    u_in = np.asarray(u)
    uc = np.clip(u_in, 0, WW - 1).astype(np.int64)

    xr, yr = _round12(x), _round12(y)
    wsrc = {"wq1": q_w1, "wq2": q_w2, "wk1": k_w1, "wk2": k_w2,
            "wv1": v_w1, "wv2": v_w2}
    wt = {n: _round12(np.asarray(w, np.float32)
                      .transpose(1, 2, 3, 0).reshape(128, 9, 128))
          for n, w in wsrc.items()}
    bias6 = np.stack([
        np.asarray(q_b1, np.float32),
        np.asarray(q_b2, np.float32) * np.float32(SCALE),
        np.asarray(k_b1, np.float32), np.asarray(k_b2, np.float32),
        np.asarray(v_b1, np.float32), np.asarray(v_b2, np.float32),
    ], axis=1)                     # (128, 6)

    in_maps, slot_maps = [], []
    for c in range(NCORE):
        im, sq = _prep_core_a(xr, yr, uc, wt, bias6, c // 2, c % 2)
        in_maps.append(im)
        slot_maps.append(sq)

    nc_a = _get_nc("a")
    res_a = run_bass_kernel_spmd(nc_a, in_maps, list(range(NCORE)))
    global LAST_EXEC_NS_A, LAST_TRACE_A
    LAST_EXEC_NS_A = res_a.exec_time_ns
    LAST_TRACE_A = res_a.instructions_and_trace

    a_full = np.zeros((BB, SS, 128), np.float32)
    for c in range(NCORE):
        # a_out: [NTILE, e, TSLOT] -> [slot, e]; divide by exp-sums
        aT = np.asarray(res_a.results[c]["a_out"], np.float32)
        flat = aT.transpose(0, 2, 1).reshape(NSLOT, 128)
        sums = np.asarray(res_a.results[c]["sums_out"], np.float32)
        sums = sums.reshape(NSLOT)
        sq = slot_maps[c]
        valid = sq >= 0
        a_full[c // 2][sq[valid]] = flat[valid] / sums[valid, None]
    a_img = a_full.transpose(0, 2, 1).reshape(BB, 128, S, S)

    wpr = _round12(np.asarray(proj_w, np.float32)
                   .transpose(1, 2, 3, 0).reshape(128, 9, 128))
    bpr = np.asarray(proj_b, np.float32).reshape(128, 1)
    in_maps_b = []
    for c in range(NCORE):
        b, rh = c // 2, c % 2
        a_slab = np.zeros((128, 18, 34), np.float32)
        r0 = 16 * rh
        rlo, rhi = max(r0 - 1, 0), min(r0 + 17, S)
        a_slab[:, (rlo - (r0 - 1)):(rhi - (r0 - 1)), 1:33] = \
            _round12(a_img[b, :, rlo:rhi, :])
        in_maps_b.append({"a_slab": a_slab, "wp": wpr, "bp": bpr})

    nc_b = _get_nc("b")
    res_b = run_bass_kernel_spmd(nc_b, in_maps_b, list(range(NCORE)))
    global LAST_EXEC_NS_B, LAST_TRACE_B, LAST_EXEC_NS
    LAST_EXEC_NS_B = res_b.exec_time_ns
    LAST_TRACE_B = res_b.instructions_and_trace
    if LAST_EXEC_NS_A is not None and LAST_EXEC_NS_B is not None:
        LAST_EXEC_NS = LAST_EXEC_NS_A + LAST_EXEC_NS_B
    return z_from_b(res_b)


def z_from_b(res_b):
    z = np.zeros((BB, 128, S, S), np.float32)
    for c in range(NCORE):
        b, rh = c // 2, c % 2
        z[b, :, 16 * rh:16 * rh + 16, :] = \
            res_b.results[c]["z_out"].reshape(128, 16, 32)
    return z
